# revision 14
# baseline (speedup 1.0000x reference)
"""ChebyKAN layer on 8 Trainium2 NeuronCores (data-parallel over batch).

Computation:  out[b,o] = sum_{i,d} T_d(tanh(x)[b,i]) * C[i,o,d]
  - batch 32768 sharded 8 ways (4096 rows/core), coefficients replicated.
  - Per core: x-shard pre-transposed on host to [i=512, b=4096] so Chebyshev
    tiles sit [i partitions, b free]; PE contracts over (i,d) with cheby tiles
    as the stationary operand and C chunks [i,o] as the moving operand,
    accumulating out[b_tile=128, o=512] in PSUM over 33 chunks of 128.
  - d=0 (T_0 == 1) is folded: its four i-chunks are pre-summed on host into a
    single [128,512] chunk matmul'd against a constant ones tile.
"""

import os
from functools import lru_cache

import numpy as np
import ml_dtypes

import concourse.bass as bass
import concourse.mybir as mybir
import concourse.tile as tile
from concourse import bacc
from concourse.bass_utils import run_bass_kernel_spmd

N_CORES = 8
BATCH, IN_F, OUT_F, DEG = 32768, 512, 512, 8
B_LOC = BATCH // N_CORES  # 4096
P = 128
N_ICHUNK = IN_F // P  # 4
N_KCHUNK = 1 + DEG * N_ICHUNK  # 33 (1 folded d=0 chunk + 32)

MM_DT_NAME = os.environ.get("CHEBY_MM_DT", "bf16")
_DT = {
    "bf16": (mybir.dt.bfloat16, ml_dtypes.bfloat16),
    "f32": (mybir.dt.float32, np.float32),
    "f32r": (mybir.dt.float32r, np.float32),
}
MM_DT, MM_NP = _DT[MM_DT_NAME]
# block of batch columns processed per iteration (SBUF-resident cheby tiles)
BBLK = 512 if MM_DT_NAME == "bf16" else 256


def _build_kernel(reps=1):
    f32 = mybir.dt.float32
    nc = bacc.Bacc(
        "TRN2",
        target_bir_lowering=False,
        debug=False,
        num_devices=N_CORES,
    )
    xT = nc.declare_dram_parameter("xT", [IN_F, B_LOC], f32, isOutput=False)
    cw = nc.declare_dram_parameter("Cw", [N_KCHUNK * P, OUT_F], MM_DT, isOutput=False)
    out = nc.declare_dram_parameter("out", [B_LOC, OUT_F], f32, isOutput=True)

    xT_ap = xT[:, :].rearrange("(c p) b -> p c b", p=P)  # [128, 4, B_LOC]
    cw_ap = cw[:, :].rearrange("(k p) o -> p k o", p=P)  # [128, 33, 512]

    import contextlib

    with tile.TileContext(nc) as tc:
        with (
            tc.tile_pool(name="const", bufs=1) as const_pool,
            tc.tile_pool(name="xin", bufs=3) as xin_pool,
            tc.tile_pool(name="tf32", bufs=1) as f32_pool,
            tc.tile_pool(name="cheb", bufs=2) as cheb_pool,
            tc.tile_pool(name="ot", bufs=4) as out_pool,
            tc.tile_pool(name="ps", bufs=6, space="PSUM") as psum_pool,
        ):
            c_tile = const_pool.tile([P, N_KCHUNK, OUT_F], MM_DT)
            # split the C load so early k-chunks land before the first matmuls
            nsplit = 4
            per = (N_KCHUNK + nsplit - 1) // nsplit
            for s in range(nsplit):
                k0, k1 = s * per, min((s + 1) * per, N_KCHUNK)
                nc.gpsimd.dma_start(
                    out=c_tile[:, k0:k1, :], in_=cw_ap[:, k0:k1, :]
                )
            ones = const_pool.tile([P, P], MM_DT)
            nc.vector.memset(ones[:, :], 1.0)

            rep_ctx = tc.For_i(0, reps, 1) if reps > 1 else contextlib.nullcontext()
            with rep_ctx:
                _kernel_body(nc, tc, xT_ap, c_tile, ones, out,
                             xin_pool, f32_pool, cheb_pool, out_pool, psum_pool)
    nc.compile()
    return nc


def _kernel_body(nc, tc, xT_ap, c_tile, ones, out,
                 xin_pool, f32_pool, cheb_pool, out_pool, psum_pool):
    f32 = mybir.dt.float32
    MULT = mybir.AluOpType.mult
    ACT_F = mybir.ActivationFunctionType

    def stt(o, a, b):  # o = 2*a*b
        nc.vector.scalar_tensor_tensor(
            out=o, in0=a, scalar=2.0, in1=b, op0=MULT, op1=MULT
        )

    def sub1(o):  # o -= 1
        nc.vector.tensor_scalar(
            out=o, in0=o, scalar1=1.0, scalar2=None,
            op0=mybir.AluOpType.subtract,
        )

    for blk in range(B_LOC // BBLK):
        b0 = blk * BBLK
        x_in = xin_pool.tile([P, N_ICHUNK, BBLK], f32)
        nc.sync.dma_start(out=x_in[:, :, :], in_=xT_ap[:, :, b0 : b0 + BBLK])

        # Tf[:, j] = T_{j+1} in fp32 (j=0..3); Tb[:, j] = T_{j+1} in bf16 (j=0..7)
        Tf = f32_pool.tile([P, 4, N_ICHUNK, BBLK], f32)
        Tb = cheb_pool.tile([P, DEG, N_ICHUNK, BBLK], MM_DT)
        t1, t2, t3, t4 = (Tf[:, j, :, :] for j in range(4))
        nc.scalar.activation(out=t1, in_=x_in[:, :, :], func=ACT_F.Tanh)
        # fp32 chain: T2=2T1^2-1, T3=2T2T1-T1, T4=2T2^2-1
        stt(t2, t1, t1); sub1(t2)
        stt(t3, t2, t1); nc.vector.tensor_sub(t3, t3, t1)
        stt(t4, t2, t2); sub1(t4)
        # one-time rounding to bf16 on the scalar engine
        for j in range(4):
            nc.scalar.copy(out=Tb[:, j, :, :], in_=Tf[:, j, :, :])
        b1, b2, b3, b4 = (Tb[:, j, :, :] for j in range(4))
        b5, b6, b7, b8 = (Tb[:, j, :, :] for j in range(4, 8))
        # bf16 products: T5=2T3T2-T1, T6=2T3^2-1, T7=2T4T3-T1, T8=2T4^2-1
        stt(b5, b3, b2); nc.vector.tensor_sub(b5, b5, b1)
        stt(b6, b3, b3); sub1(b6)
        stt(b7, b4, b3); nc.vector.tensor_sub(b7, b7, b1)
        stt(b8, b4, b4); sub1(b8)

        for bt in range(BBLK // P):
            psum = psum_pool.tile([P, OUT_F], f32, space="PSUM")
            # k=0: folded d=0 chunk against constant ones
            nc.tensor.matmul(
                psum[:, :], ones[:, :], c_tile[:, 0, :], start=True, stop=False
            )
            for j in range(DEG):
                for c in range(N_ICHUNK):
                    k = 1 + j * N_ICHUNK + c
                    nc.tensor.matmul(
                        psum[:, :],
                        Tb[:, j, c, bt * P : (bt + 1) * P],
                        c_tile[:, k, :],
                        start=False,
                        stop=(k == N_KCHUNK - 1),
                    )
            o_tile = out_pool.tile([P, OUT_F], f32)
            nc.scalar.copy(out=o_tile[:, :], in_=psum[:, :])
            row = b0 + bt * P
            nc.sync.dma_start(out=out[row : row + P, :], in_=o_tile[:, :])


@lru_cache(maxsize=4)
def _get_nc(reps=1):
    return _build_kernel(reps)


class Runner:
    """Persistent jitted runner mirroring bass2jax.run_bass_via_pjrt, reusable
    across calls (single jit cache entry) so repeated executions can be timed
    back-to-back without recompilation or host round-trips per call."""

    def __init__(self, nc):
        import jax
        import jax.numpy as jnp
        from jax.sharding import Mesh, PartitionSpec
        from jax.experimental.shard_map import shard_map
        from concourse import bass2jax
        from concourse import mybir as _mybir

        bass2jax.install_neuronx_cc_hook()
        self.jax = jax
        self.nc = nc
        partition_name = (
            nc.partition_id_tensor.name if nc.partition_id_tensor else None
        )
        in_names, out_names, out_avals = [], [], []
        for alloc in nc.m.functions[0].allocations:
            if not isinstance(alloc, _mybir.MemoryLocationSet):
                continue
            name = alloc.memorylocations[0].name
            if alloc.kind == "ExternalInput":
                if name != partition_name:
                    in_names.append(name)
            elif alloc.kind == "ExternalOutput":
                out_names.append(name)
                out_avals.append(
                    jax.core.ShapedArray(
                        tuple(alloc.tensor_shape), _mybir.dt.np(alloc.dtype)
                    )
                )
        self.in_names = list(in_names)
        self.out_names = out_names
        self.out_avals = out_avals
        n_params = len(in_names)
        all_names = in_names + out_names
        if partition_name is not None:
            all_names = all_names + [partition_name]

        def _body(*args):
            operands = list(args)
            if partition_name is not None:
                operands.append(bass2jax.partition_id_tensor())
            return tuple(
                bass2jax._bass_exec_p.bind(
                    *operands,
                    out_avals=tuple(out_avals),
                    in_names=tuple(all_names),
                    out_names=tuple(out_names),
                    lowering_input_output_aliases=(),
                    sim_require_finite=True,
                    sim_require_nnan=True,
                    nc=nc,
                )
            )

        devices = jax.devices()[:N_CORES]
        self.mesh = Mesh(np.asarray(devices), ("core",))
        in_specs = (PartitionSpec("core"),) * (n_params + len(out_names))
        out_specs = (PartitionSpec("core"),) * len(out_names)
        self.fn = jax.jit(
            shard_map(
                _body,
                mesh=self.mesh,
                in_specs=in_specs,
                out_specs=out_specs,
                check_rep=False,
            ),
            keep_unused=True,
        )

    def put_inputs(self, in_maps):
        import jax
        from jax.sharding import NamedSharding, PartitionSpec

        concat = [
            np.concatenate([np.asarray(m[name]) for m in in_maps], axis=0)
            for name in self.in_names
        ]
        for aval in self.out_avals:
            concat.append(
                np.zeros((N_CORES * aval.shape[0], *aval.shape[1:]), aval.dtype)
            )
        sh = NamedSharding(self.mesh, PartitionSpec("core"))
        return [jax.device_put(a, sh) for a in concat]

    def __call__(self, dev_inputs):
        return self.fn(*dev_inputs)

    def run_np(self, in_maps):
        outs = self(self.put_inputs(in_maps))
        return [
            {
                name: np.asarray(outs[i]).reshape(N_CORES, *self.out_avals[i].shape)[c]
                for i, name in enumerate(self.out_names)
            }
            for c in range(N_CORES)
        ]


def _prep_inputs(x: np.ndarray, coefficients: np.ndarray):
    x = np.asarray(x, dtype=np.float32)
    coefficients = np.asarray(coefficients, dtype=np.float32)
    # C chunks: k=0 is the d=0 term pre-summed over its 4 i-chunks (T_0 == 1);
    # k=1+j*4+c is degree j+1, i-chunk c, laid out [i within chunk, o].
    c_perm = np.transpose(coefficients, (2, 0, 1))  # (d, i, o)
    c0 = c_perm[0].reshape(N_ICHUNK, P, OUT_F).sum(axis=0)  # (128, 512)
    c_main = c_perm[1:].reshape(DEG * N_ICHUNK, P, OUT_F)
    c_all = np.concatenate([c0[None], c_main], axis=0)
    c_all = np.ascontiguousarray(c_all.reshape(N_KCHUNK * P, OUT_F)).astype(MM_NP)

    in_maps = []
    for core in range(N_CORES):
        shard = x[core * B_LOC : (core + 1) * B_LOC]  # (4096, 512)
        xt = np.ascontiguousarray(shard.T)  # (512, 4096)
        in_maps.append({"xT": xt, "Cw": c_all})
    return in_maps


@lru_cache(maxsize=4)
def _get_runner(reps=1):
    return Runner(_get_nc(reps))


def run_sharded(x, coefficients):
    """Run the 8-core kernel; returns the full (32768, 512) float32 output."""
    in_maps = _prep_inputs(x, coefficients)
    runner = _get_runner()
    results = runner.run_np(in_maps)
    parts = [np.asarray(results[i]["out"]) for i in range(N_CORES)]
    return np.concatenate(parts, axis=0).astype(np.float32)


def _time_runner(runner, dev_in, iters):
    import time

    outs = runner(dev_in)  # warm up
    outs[0].block_until_ready()
    times = []
    for _ in range(iters):
        t0 = time.perf_counter()
        outs = runner(dev_in)
        outs[0].block_until_ready()
        times.append((time.perf_counter() - t0) * 1e9)
    return times


def bench(x, coefficients, iters=15, rep_a=3, rep_b=43):
    """Estimate per-invocation HW time from the slope between two on-device
    repeat counts (fixed ~94ms axon RPC overhead cancels). Returns
    (slope_ns, times_a, times_b)."""
    in_maps = _prep_inputs(x, coefficients)
    ra, rb = _get_runner(rep_a), _get_runner(rep_b)
    dev_a = ra.put_inputs(in_maps)
    dev_b = rb.put_inputs(in_maps)
    ta = _time_runner(ra, dev_a, iters)
    tb = _time_runner(rb, dev_b, iters)
    slope = (min(tb) - min(ta)) / (rep_b - rep_a)
    return slope, ta, tb


def kernel(x, coefficients):
    return run_sharded(x, coefficients)
